# revision 30
# baseline (speedup 1.0000x reference)
"""Trainium2 Bass kernel for DirectSolverNet (Direct-Nodamping, inverse).

Math per batch element b:
    wR   = (weights * R).reshape(-1)                 # [N], N = 8*120*160
    JtR  = Jt[b] @ wR                                # [6]
    H    = JtJ[b] + 1e-6*trace(JtJ[b]) * I6
    xi   = H^-1 @ JtR                                # [6]
    dR   = rodrigues(-xi[:3]); dt = -(dR @ xi[3:])
    R_new = pose_R @ dR;  t_new = pose_R @ dt + pose_t

Sharding: pure batch parallel, 8 batches per NeuronCore across 8 cores.

Per-core device strategy (memory-bound; ~39 MB HBM traffic per core):
  - Stream weights/R/Jt as [128, 1200]-shaped fp32 tiles (p-major layout of
    each contiguous 153600-float row).
  - wR via one DVE tensor_tensor multiply per batch.
  - JtR row-dots via the fused DVE tensor_tensor_reduce (one instruction per
    Jt row: multiply + free-dim reduce) -> per-partition partials [128,1].
  - Partition reduction of the [128, 48] partials with 6 tiny TensorE
    matmuls against a ones vector -> JtR as PSUM [8, 6].
  - 6x6 inverse via Gauss-Jordan on the augmented [H | I] laid out as
    [8 partitions, 72] (batch on partitions), overlapped with streaming.
  - Rodrigues / 3x3 composes as batched strided DVE/ACT ops on [8, k] tiles.
"""

import sys

sys.path.insert(0, "/opt/trn_rl_repo")

import math

import numpy as np

B = 64
N_CORES = 8
B_PER = B // N_CORES  # 8 batches per core
C, H, W = 8, 120, 160
N = C * H * W  # 153600
P = 128
F = N // P  # 1200

_CACHE: dict = {}


def _build_module():
    import concourse.bacc as bacc
    import concourse.tile as tile
    from concourse import mybir

    f32 = mybir.dt.float32
    Al = mybir.AluOpType
    Act = mybir.ActivationFunctionType

    nc = bacc.Bacc(
        "TRN2",
        target_bir_lowering=False,
        debug=False,
        enable_asserts=False,
        num_devices=N_CORES,
    )

    jt_d = nc.dram_tensor("jt", [B_PER, 6, N], f32, kind="ExternalInput").ap()
    w_d = nc.dram_tensor("w", [B_PER, N], f32, kind="ExternalInput").ap()
    r_d = nc.dram_tensor("r", [B_PER, N], f32, kind="ExternalInput").ap()
    jtj_d = nc.dram_tensor("jtj", [B_PER, 36], f32, kind="ExternalInput").ap()
    pr_d = nc.dram_tensor("pr", [B_PER, 9], f32, kind="ExternalInput").ap()
    pt_d = nc.dram_tensor("pt", [B_PER, 3], f32, kind="ExternalInput").ap()
    eye6_d = nc.dram_tensor("eye6", [B_PER, 36], f32, kind="ExternalInput").ap()
    eye3_d = nc.dram_tensor("eye3", [B_PER, 9], f32, kind="ExternalInput").ap()
    out_d = nc.dram_tensor("out12", [B_PER, 12], f32, kind="ExternalOutput").ap()

    with tile.TileContext(nc) as tc:
        with (
            tc.tile_pool(name="single", bufs=1) as single,
            tc.tile_pool(name="small", bufs=3) as small,
            tc.tile_pool(name="wrp", bufs=2) as wrp,
            tc.tile_pool(name="jtp", bufs=6) as jtp,
            tc.tile_pool(name="psum", bufs=1, space="PSUM") as psum,
        ):
            v = nc.vector
            sc = nc.scalar

            # ---- constants + small inputs -------------------------------
            ones = single.tile([P, 1], f32)
            v.memset(ones[:], 1.0)
            # tiny first DMA on the SP ring: absorbs the HWDGE ring bring-up
            # latency so the first 3.6MB jt transfer streams immediately.
            jtjt = single.tile([B_PER, 36], f32)
            nc.sync.dma_start(jtjt[:], jtj_d)
            eye6t = single.tile([B_PER, 36], f32)
            nc.scalar.dma_start(eye6t[:], eye6_d)
            eye3t = single.tile([B_PER, 9], f32)
            nc.scalar.dma_start(eye3t[:], eye3_d)
            prt = single.tile([B_PER, 9], f32)
            nc.scalar.dma_start(prt[:], pr_d)
            ptt = single.tile([B_PER, 3], f32)
            nc.scalar.dma_start(ptt[:], pt_d)

            acc = single.tile([P, 6 * B_PER], f32)  # JtR partials, col = k*8+b
            # spill column for the split tail of the very last row (b=7,k=5):
            # col 7 gets the last 200-col partial, other cols stay zero.
            accB = single.tile([P, B_PER], f32)
            v.memset(accB[:], 0.0)

            # Leave the Sqrt LUT resident on ScalarE: the ACT table reloads on
            # each function switch (~1.3us), so end the warmup with Sqrt - the
            # first tail activation is Sqrt and skips its load.
            zero1 = single.tile([B_PER, 1], f32)
            v.memset(zero1[:], 0.0)
            warm1 = single.tile([B_PER, 1], f32)
            sc.activation(warm1[:], zero1[:], Act.Sin, bias=zero1[:])
            sc.activation(warm1[:], zero1[:], Act.Sqrt, bias=zero1[:])

            # ---- H = JtJ + 1e-6*trace*I, then Gauss-Jordan inverse ------
            tr = single.tile([B_PER, 1], f32)
            scr36 = single.tile([B_PER, 36], f32)
            v.scalar_tensor_tensor(scr36[:], jtjt[:], 1.0, eye6t[:],
                                   Al.mult, Al.mult, accum_out=tr[:])
            damp = single.tile([B_PER, 1], f32)
            v.tensor_scalar(damp[:], tr[:], 1e-6, None, Al.mult)
            # scr36 = damp * I6
            v.tensor_scalar(scr36[:], eye6t[:], damp[:], None, Al.mult)

            aug = single.tile([B_PER, 72], f32)  # [H | I], row-major 6x12
            augv = aug[:].rearrange("b (i j) -> b i j", i=6)
            e6v = eye6t[:].rearrange("b (i j) -> b i j", i=6)
            jv = jtjt[:].rearrange("b (i j) -> b i j", i=6)
            sv36 = scr36[:].rearrange("b (i j) -> b i j", i=6)
            v.tensor_tensor(augv[:, :, 0:6], jv, sv36, Al.add)
            v.tensor_copy(augv[:, :, 6:12], e6v)

            for k in range(6):
                rcp = small.tile([B_PER, 1], f32, tag="rcp")
                piv = augv[:, k : k + 1, k : k + 1].rearrange("b i j -> b (i j)")
                v.reciprocal(rcp[:], piv)
                rowk = small.tile([B_PER, 12], f32, tag="rowk")
                v.tensor_scalar(rowk[:], aug[:, 12 * k : 12 * k + 12], rcp[:], None, Al.mult)
                delta = small.tile([B_PER, 72], f32, tag="delta")
                colk = augv[:, :, k : k + 1].broadcast_to([B_PER, 6, 12])
                rowb = rowk[:].unsqueeze(1).broadcast_to([B_PER, 6, 12])
                v.tensor_tensor(delta[:].rearrange("b (i j) -> b i j", i=6), colk, rowb, Al.mult)
                v.tensor_sub(aug[:], aug[:], delta[:])
                v.tensor_copy(aug[:, 12 * k : 12 * k + 12], rowk[:])

            # ---- streaming: wR and fused JtR row dots -------------------
            jt_view = jt_d.rearrange("b k (p f) -> b k p f", p=P)
            w_view = w_d.rearrange("b (p f) -> b p f", p=P)
            r_view = r_d.rearrange("b (p f) -> b p f", p=P)
            for b in range(B_PER):
                jt = jtp.tile([P, 6 * F], f32, tag="jt")

                def _load_jt(jt=jt, b=b):
                    if b < B_PER - 1:
                        # one 3.6MB DMA per batch (fewer serial issue slots on
                        # SP); partition-first 3D APs on both sides.
                        nc.sync.dma_start(
                            jt[:].rearrange("p (k f) -> p k f", k=6),
                            jt_d[b].rearrange("k (p f) -> p k f", p=P),
                        )
                    else:
                        # last batch: per-row DMAs so the tail only waits for
                        # the final 600KB row, not the whole 3.6MB batch; the
                        # very last row is split so only a 200-col stub sits
                        # behind the final DMA-completion latency.
                        for k in range(5):
                            nc.sync.dma_start(jt[:, k * F : (k + 1) * F], jt_view[b, k])
                        nc.sync.dma_start(jt[:, 5 * F : 5 * F + 1000], jt_view[b, 5][:, 0:1000])
                        nc.sync.dma_start(jt[:, 5 * F + 1000 : 6 * F], jt_view[b, 5][:, 1000:F])

                if b == 0:
                    _load_jt()  # get the big stream going first
                wt = wrp.tile([P, F], f32, tag="wt")
                nc.sync.dma_start(wt[:], w_view[b])
                rt = wrp.tile([P, F], f32, tag="rt")
                nc.sync.dma_start(rt[:], r_view[b])
                wr = wrp.tile([P, F], f32, tag="wr")
                v.tensor_mul(wr[:], wt[:], rt[:])
                if b > 0:
                    _load_jt()
                for k in range(6):
                    if b == B_PER - 1 and k == 5:
                        sl = jt[:, 5 * F : 5 * F + 1000]
                        v.scalar_tensor_tensor(
                            sl, sl, 1.0, wr[:, 0:1000], Al.mult, Al.mult,
                            accum_out=acc[:, k * B_PER + b : k * B_PER + b + 1],
                        )
                        sl = jt[:, 5 * F + 1000 : 6 * F]
                        v.scalar_tensor_tensor(
                            sl, sl, 1.0, wr[:, 1000:F], Al.mult, Al.mult,
                            accum_out=accB[:, b : b + 1],
                        )
                    else:
                        sl = jt[:, k * F : (k + 1) * F]
                        v.scalar_tensor_tensor(
                            sl, sl, 1.0, wr[:], Al.mult, Al.mult,
                            accum_out=acc[:, k * B_PER + b : k * B_PER + b + 1],
                        )

            # ---- partition-reduce the partials on TensorE ---------------
            psum_bk = psum.tile([B_PER, 6], f32)
            for k in range(6):
                last = k == 5
                nc.tensor.matmul(
                    psum_bk[:, k : k + 1],
                    acc[:, k * B_PER : (k + 1) * B_PER],
                    ones[:],
                    start=True,
                    stop=not last,
                )
                if last:
                    nc.tensor.matmul(
                        psum_bk[:, k : k + 1], accB[:], ones[:],
                        start=False, stop=True,
                    )
            jtr = single.tile([B_PER, 6], f32)
            sc.copy(jtr[:], psum_bk[:])

            # ---- xi = Hinv @ JtR ---------------------------------------
            hinv = augv[:, :, 6:12]  # [8, i, j]
            p36 = single.tile([B_PER, 36], f32)
            jtr_b = jtr[:].unsqueeze(1).broadcast_to([B_PER, 6, 6])
            v.tensor_tensor(p36[:].rearrange("b (i j) -> b i j", i=6), hinv, jtr_b, Al.mult)
            xi = single.tile([B_PER, 6], f32)
            v.tensor_reduce(
                xi[:], p36[:].rearrange("b (i j) -> b i j", i=6),
                axis=mybir.AxisListType.X, op=Al.add,
            )

            # ---- Rodrigues dR = rod(-xi[:3]) ---------------------------
            wv = single.tile([B_PER, 3], f32)
            v.tensor_scalar(wv[:], xi[:, 0:3], -1.0, None, Al.mult)
            sq = single.tile([B_PER, 3], f32)
            v.tensor_mul(sq[:], wv[:], wv[:])
            s2 = single.tile([B_PER, 1], f32)
            v.tensor_reduce(s2[:], sq[:], axis=mybir.AxisListType.X, op=Al.add)
            th = single.tile([B_PER, 1], f32)
            sc.activation(th[:], s2[:], Act.Sqrt, bias=zero1[:])
            v.tensor_scalar(th[:], th[:], 1e-12, None, Al.max)
            rth = single.tile([B_PER, 1], f32)
            v.reciprocal(rth[:], th[:])
            u = single.tile([B_PER, 3], f32)
            v.tensor_scalar(u[:], wv[:], rth[:], None, Al.mult)
            # Pack [theta, theta + pi/2] into one [8,2] tile, range-reduce into
            # [-pi, pi] (Sin LUT domain; y -> y - 2pi*(y>pi) twice, since theta
            # can reach ~10), then a single Sin gives sin and cos together.
            th2 = single.tile([B_PER, 2], f32)
            v.tensor_copy(th2[:, 0:1], th[:])
            v.tensor_scalar(th2[:, 1:2], th[:], math.pi / 2, None, Al.add)
            mask1 = single.tile([B_PER, 2], f32)
            for _ in range(2):
                v.tensor_scalar(mask1[:], th2[:], math.pi, None, Al.is_gt)
                v.scalar_tensor_tensor(th2[:], mask1[:], -2.0 * math.pi, th2[:],
                                       Al.mult, Al.add)
            sc2 = single.tile([B_PER, 2], f32)
            sc.activation(sc2[:], th2[:], Act.Sin, bias=zero1[:])
            sint = sc2[:, 0:1]
            cost = sc2[:, 1:2]
            omc = single.tile([B_PER, 1], f32)
            v.tensor_scalar(omc[:], cost[:], -1.0, 1.0, Al.mult, Al.add)

            uu = single.tile([B_PER, 9], f32)
            u_i = u[:].unsqueeze(2).broadcast_to([B_PER, 3, 3])
            u_j = u[:].unsqueeze(1).broadcast_to([B_PER, 3, 3])
            v.tensor_tensor(uu[:].rearrange("b (i j) -> b i j", i=3), u_i, u_j, Al.mult)

            kk = single.tile([B_PER, 9], f32)
            v.memset(kk[:], 0.0)
            for col, (src, sgn) in {1: (2, -1.0), 2: (1, 1.0), 3: (2, 1.0),
                                    5: (0, -1.0), 6: (1, -1.0), 7: (0, 1.0)}.items():
                v.tensor_scalar(kk[:, col : col + 1], u[:, src : src + 1], sgn, None, Al.mult)

            t9 = single.tile([B_PER, 9], f32)
            v.tensor_scalar(t9[:], uu[:], omc[:], None, Al.mult)
            dr = single.tile([B_PER, 9], f32)
            v.scalar_tensor_tensor(dr[:], kk[:], sint[:], t9[:], Al.mult, Al.add)
            v.scalar_tensor_tensor(dr[:], eye3t[:], cost[:], dr[:], Al.mult, Al.add)

            # ---- dtv = dR @ xi[3:]; R_new = pR@dR; t_new = pt - pR@dtv --
            p9 = single.tile([B_PER, 9], f32)
            drv = dr[:].rearrange("b (i j) -> b i j", i=3)
            xi2 = xi[:, 3:6].unsqueeze(1).broadcast_to([B_PER, 3, 3])
            v.tensor_tensor(p9[:].rearrange("b (i j) -> b i j", i=3), drv, xi2, Al.mult)
            dtv = single.tile([B_PER, 3], f32)
            v.tensor_reduce(dtv[:], p9[:].rearrange("b (i j) -> b i j", i=3),
                            axis=mybir.AxisListType.X, op=Al.add)

            p27 = single.tile([B_PER, 27], f32)
            pr_v = prt[:].rearrange("b (i j) -> b i j", i=3).unsqueeze(2).broadcast_to([B_PER, 3, 3, 3])
            dr_v = dr[:].rearrange("b (j k) -> b k j", j=3).unsqueeze(1).broadcast_to([B_PER, 3, 3, 3])
            v.tensor_tensor(p27[:].rearrange("b (i k j) -> b i k j", i=3, k=3), pr_v, dr_v, Al.mult)
            out12 = single.tile([B_PER, 12], f32)
            v.tensor_reduce(out12[:, 0:9], p27[:].rearrange("b (i k j) -> b i k j", i=3, k=3),
                            axis=mybir.AxisListType.X, op=Al.add)

            prv3 = prt[:].rearrange("b (i j) -> b i j", i=3)
            dtb = dtv[:].unsqueeze(1).broadcast_to([B_PER, 3, 3])
            v.tensor_tensor(p9[:].rearrange("b (i j) -> b i j", i=3), prv3, dtb, Al.mult)
            tmp3 = single.tile([B_PER, 3], f32)
            v.tensor_reduce(tmp3[:], p9[:].rearrange("b (i j) -> b i j", i=3),
                            axis=mybir.AxisListType.X, op=Al.add)
            v.tensor_sub(out12[:, 9:12], ptt[:], tmp3[:])

            nc.sync.dma_start(out_d, out12[:])

    nc.compile()
    return nc


def _get_nc():
    if "nc" not in _CACHE:
        _CACHE["nc"] = _build_module()
    return _CACHE["nc"]


def _make_in_maps(JtJ, Jt, weights, R, pose_R, pose_t):
    eye6 = np.broadcast_to(np.eye(6, dtype=np.float32).reshape(1, 36), (B_PER, 36)).copy()
    eye3 = np.broadcast_to(np.eye(3, dtype=np.float32).reshape(1, 9), (B_PER, 9)).copy()
    in_maps = []
    for c in range(N_CORES):
        s = slice(c * B_PER, (c + 1) * B_PER)
        in_maps.append(
            {
                "jt": np.ascontiguousarray(Jt[s], dtype=np.float32),
                "w": np.ascontiguousarray(weights[s], dtype=np.float32).reshape(B_PER, N),
                "r": np.ascontiguousarray(R[s], dtype=np.float32).reshape(B_PER, N),
                "jtj": np.ascontiguousarray(JtJ[s], dtype=np.float32).reshape(B_PER, 36),
                "pr": np.ascontiguousarray(pose_R[s], dtype=np.float32).reshape(B_PER, 9),
                "pt": np.ascontiguousarray(pose_t[s], dtype=np.float32),
                "eye6": eye6,
                "eye3": eye3,
            }
        )
    return in_maps


def run(inputs, trace=False, **kw):
    """Run the bass kernel on the 8 cores. Returns ((R_new, t_new), results)."""
    from concourse import bass_utils

    nc = _get_nc()
    in_maps = _make_in_maps(
        inputs["JtJ"], inputs["Jt"], inputs["weights"], inputs["R"],
        inputs["pose_R"], inputs["pose_t"],
    )
    res = bass_utils.run_bass_kernel_spmd(
        nc, in_maps, core_ids=list(range(N_CORES)), trace=trace, **kw
    )
    out = np.concatenate([res.results[c]["out12"] for c in range(N_CORES)])
    rn = np.ascontiguousarray(out[:, 0:9], dtype=np.float32).reshape(B, 3, 3)
    tn = np.ascontiguousarray(out[:, 9:12], dtype=np.float32)
    return (rn, tn), res


def kernel(JtJ, Jt, weights, R, pose_R, pose_t, invD0=None, invD1=None,
           x0=None, x1=None, K=None):
    (rn, tn), _ = run(dict(JtJ=JtJ, Jt=Jt, weights=weights, R=R,
                           pose_R=pose_R, pose_t=pose_t))
    return rn, tn


# revision 31
# speedup vs baseline: 1.1517x; 1.1517x over previous
"""Trainium2 Bass kernel for DirectSolverNet (Direct-Nodamping, inverse).

Math per batch element b:
    wR   = (weights * R).reshape(-1)                 # [N], N = 8*120*160
    JtR  = Jt[b] @ wR                                # [6]
    H    = JtJ[b] + 1e-6*trace(JtJ[b]) * I6
    xi   = H^-1 @ JtR                                # [6]
    dR   = rodrigues(-xi[:3]); dt = -(dR @ xi[3:])
    R_new = pose_R @ dR;  t_new = pose_R @ dt + pose_t

Sharding: pure batch parallel, 8 batches per NeuronCore across 8 cores.

Per-core device strategy (memory-bound; ~39 MB HBM traffic per core):
  - Stream weights/R/Jt as [128, 1200]-shaped fp32 tiles (p-major layout of
    each contiguous 153600-float row).
  - wR via one DVE tensor_tensor multiply per batch.
  - JtR row-dots via the fused DVE tensor_tensor_reduce (one instruction per
    Jt row: multiply + free-dim reduce) -> per-partition partials [128,1].
  - Partition reduction of the [128, 48] partials with 6 tiny TensorE
    matmuls against a ones vector -> JtR as PSUM [8, 6].
  - 6x6 inverse via Gauss-Jordan on the augmented [H | I] laid out as
    [8 partitions, 72] (batch on partitions), overlapped with streaming.
  - Rodrigues / 3x3 composes as batched strided DVE/ACT ops on [8, k] tiles.
"""

import sys

sys.path.insert(0, "/opt/trn_rl_repo")

import math

import numpy as np

B = 64
N_CORES = 8
B_PER = B // N_CORES  # 8 batches per core
C, H, W = 8, 120, 160
N = C * H * W  # 153600
P = 128
F = N // P  # 1200

_CACHE: dict = {}


def _build_module():
    import concourse.bacc as bacc
    import concourse.tile as tile
    from concourse import mybir

    f32 = mybir.dt.float32
    Al = mybir.AluOpType
    Act = mybir.ActivationFunctionType

    nc = bacc.Bacc(
        "TRN2",
        target_bir_lowering=False,
        debug=False,
        enable_asserts=False,
        num_devices=N_CORES,
    )

    jt_d = nc.dram_tensor("jt", [B_PER, 6, N], f32, kind="ExternalInput").ap()
    w_d = nc.dram_tensor("w", [B_PER, N], f32, kind="ExternalInput").ap()
    r_d = nc.dram_tensor("r", [B_PER, N], f32, kind="ExternalInput").ap()
    jtj_d = nc.dram_tensor("jtj", [B_PER, 36], f32, kind="ExternalInput").ap()
    pr_d = nc.dram_tensor("pr", [B_PER, 9], f32, kind="ExternalInput").ap()
    pt_d = nc.dram_tensor("pt", [B_PER, 3], f32, kind="ExternalInput").ap()
    eye6_d = nc.dram_tensor("eye6", [B_PER, 36], f32, kind="ExternalInput").ap()
    eye3_d = nc.dram_tensor("eye3", [B_PER, 9], f32, kind="ExternalInput").ap()
    out_d = nc.dram_tensor("out12", [B_PER, 12], f32, kind="ExternalOutput").ap()

    with tile.TileContext(nc) as tc:
        with (
            tc.tile_pool(name="single", bufs=1) as single,
            tc.tile_pool(name="small", bufs=3) as small,
            tc.tile_pool(name="wrp", bufs=2) as wrp,
            tc.tile_pool(name="jtp", bufs=5) as jtp,
            tc.tile_pool(name="psum", bufs=1, space="PSUM") as psum,
        ):
            v = nc.vector
            sc = nc.scalar

            # ---- constants + small inputs -------------------------------
            ones = single.tile([P, 1], f32)
            v.memset(ones[:], 1.0)
            # tiny first DMA on the SP ring: absorbs the HWDGE ring bring-up
            # latency so the first 3.6MB jt transfer streams immediately.
            jtjt = single.tile([B_PER, 36], f32)
            nc.sync.dma_start(jtjt[:], jtj_d)
            eye6t = single.tile([B_PER, 36], f32)
            nc.scalar.dma_start(eye6t[:], eye6_d)
            eye3t = single.tile([B_PER, 9], f32)
            nc.scalar.dma_start(eye3t[:], eye3_d)
            prt = single.tile([B_PER, 9], f32)
            nc.scalar.dma_start(prt[:], pr_d)
            ptt = single.tile([B_PER, 3], f32)
            nc.scalar.dma_start(ptt[:], pt_d)

            acc = single.tile([P, 6 * B_PER], f32)  # JtR partials, col = k*8+b
            # spill column for the split tail of the very last row (b=7,k=5):
            # col 7 gets the last 200-col partial, other cols stay zero.
            accB = single.tile([P, B_PER], f32)
            v.memset(accB[:], 0.0)

            # Leave the Sqrt LUT resident on ScalarE: the ACT table reloads on
            # each function switch (~1.3us), so end the warmup with Sqrt - the
            # first tail activation is Sqrt and skips its load.
            zero1 = single.tile([B_PER, 1], f32)
            v.memset(zero1[:], 0.0)
            warm1 = single.tile([B_PER, 1], f32)
            sc.activation(warm1[:], zero1[:], Act.Sin, bias=zero1[:])
            sc.activation(warm1[:], zero1[:], Act.Sqrt, bias=zero1[:])

            # ---- H = JtJ + 1e-6*trace*I, then Gauss-Jordan inverse ------
            tr = single.tile([B_PER, 1], f32)
            scr36 = single.tile([B_PER, 36], f32)
            v.scalar_tensor_tensor(scr36[:], jtjt[:], 1.0, eye6t[:],
                                   Al.mult, Al.mult, accum_out=tr[:])
            damp = single.tile([B_PER, 1], f32)
            v.tensor_scalar(damp[:], tr[:], 1e-6, None, Al.mult)
            # scr36 = damp * I6
            v.tensor_scalar(scr36[:], eye6t[:], damp[:], None, Al.mult)

            aug = single.tile([B_PER, 72], f32)  # [H | I], row-major 6x12
            augv = aug[:].rearrange("b (i j) -> b i j", i=6)
            e6v = eye6t[:].rearrange("b (i j) -> b i j", i=6)
            jv = jtjt[:].rearrange("b (i j) -> b i j", i=6)
            sv36 = scr36[:].rearrange("b (i j) -> b i j", i=6)
            v.tensor_tensor(augv[:, :, 0:6], jv, sv36, Al.add)
            v.tensor_copy(augv[:, :, 6:12], e6v)

            for k in range(6):
                rcp = small.tile([B_PER, 1], f32, tag="rcp")
                piv = augv[:, k : k + 1, k : k + 1].rearrange("b i j -> b (i j)")
                v.reciprocal(rcp[:], piv)
                rowk = small.tile([B_PER, 12], f32, tag="rowk")
                v.tensor_scalar(rowk[:], aug[:, 12 * k : 12 * k + 12], rcp[:], None, Al.mult)
                delta = small.tile([B_PER, 72], f32, tag="delta")
                colk = augv[:, :, k : k + 1].broadcast_to([B_PER, 6, 12])
                rowb = rowk[:].unsqueeze(1).broadcast_to([B_PER, 6, 12])
                v.tensor_tensor(delta[:].rearrange("b (i j) -> b i j", i=6), colk, rowb, Al.mult)
                v.tensor_sub(aug[:], aug[:], delta[:])
                v.tensor_copy(aug[:, 12 * k : 12 * k + 12], rowk[:])

            # ---- streaming: wR and fused JtR row dots -------------------
            jt_view = jt_d.rearrange("b k (p f) -> b k p f", p=P)
            w_view = w_d.rearrange("b (p f) -> b p f", p=P)
            r_view = r_d.rearrange("b (p f) -> b p f", p=P)
            for b in range(B_PER):
                jt = jtp.tile([P, 6 * F], f32, tag="jt")

                def _load_jt(jt=jt, b=b):
                    if b < B_PER - 1:
                        # one 3.6MB DMA per batch (fewer serial issue slots on
                        # SP); partition-first 3D APs on both sides.
                        nc.sync.dma_start(
                            jt[:].rearrange("p (k f) -> p k f", k=6),
                            jt_d[b].rearrange("k (p f) -> p k f", p=P),
                        )
                    else:
                        # last batch: per-row DMAs so the tail only waits for
                        # the final 600KB row, not the whole 3.6MB batch; the
                        # very last row is split so only a 200-col stub sits
                        # behind the final DMA-completion latency.
                        for k in range(5):
                            nc.sync.dma_start(jt[:, k * F : (k + 1) * F], jt_view[b, k])
                        nc.sync.dma_start(jt[:, 5 * F : 5 * F + 1000], jt_view[b, 5][:, 0:1000])
                        nc.sync.dma_start(jt[:, 5 * F + 1000 : 6 * F], jt_view[b, 5][:, 1000:F])

                if b == 0:
                    _load_jt()  # get the big stream going first
                wt = wrp.tile([P, F], f32, tag="wt")
                nc.sync.dma_start(wt[:], w_view[b])
                rt = wrp.tile([P, F], f32, tag="rt")
                nc.sync.dma_start(rt[:], r_view[b])
                wr = wrp.tile([P, F], f32, tag="wr")
                v.tensor_mul(wr[:], wt[:], rt[:])
                if b > 0:
                    _load_jt()
                for k in range(6):
                    if b == B_PER - 1 and k == 5:
                        sl = jt[:, 5 * F : 5 * F + 1000]
                        v.scalar_tensor_tensor(
                            sl, sl, 1.0, wr[:, 0:1000], Al.mult, Al.mult,
                            accum_out=acc[:, k * B_PER + b : k * B_PER + b + 1],
                        )
                        sl = jt[:, 5 * F + 1000 : 6 * F]
                        v.scalar_tensor_tensor(
                            sl, sl, 1.0, wr[:, 1000:F], Al.mult, Al.mult,
                            accum_out=accB[:, b : b + 1],
                        )
                    else:
                        sl = jt[:, k * F : (k + 1) * F]
                        v.scalar_tensor_tensor(
                            sl, sl, 1.0, wr[:], Al.mult, Al.mult,
                            accum_out=acc[:, k * B_PER + b : k * B_PER + b + 1],
                        )

            # ---- partition-reduce the partials on TensorE ---------------
            psum_bk = psum.tile([B_PER, 6], f32)
            for k in range(6):
                last = k == 5
                nc.tensor.matmul(
                    psum_bk[:, k : k + 1],
                    acc[:, k * B_PER : (k + 1) * B_PER],
                    ones[:],
                    start=True,
                    stop=not last,
                )
                if last:
                    nc.tensor.matmul(
                        psum_bk[:, k : k + 1], accB[:], ones[:],
                        start=False, stop=True,
                    )
            jtr = single.tile([B_PER, 6], f32)
            sc.copy(jtr[:], psum_bk[:])

            # ---- xi = Hinv @ JtR ---------------------------------------
            hinv = augv[:, :, 6:12]  # [8, i, j]
            p36 = single.tile([B_PER, 36], f32)
            jtr_b = jtr[:].unsqueeze(1).broadcast_to([B_PER, 6, 6])
            v.tensor_tensor(p36[:].rearrange("b (i j) -> b i j", i=6), hinv, jtr_b, Al.mult)
            xi = single.tile([B_PER, 6], f32)
            v.tensor_reduce(
                xi[:], p36[:].rearrange("b (i j) -> b i j", i=6),
                axis=mybir.AxisListType.X, op=Al.add,
            )

            # ---- Rodrigues dR = rod(-xi[:3]) ---------------------------
            wv = single.tile([B_PER, 3], f32)
            v.tensor_scalar(wv[:], xi[:, 0:3], -1.0, None, Al.mult)
            sq = single.tile([B_PER, 3], f32)
            v.tensor_mul(sq[:], wv[:], wv[:])
            s2 = single.tile([B_PER, 1], f32)
            v.tensor_reduce(s2[:], sq[:], axis=mybir.AxisListType.X, op=Al.add)
            th = single.tile([B_PER, 1], f32)
            sc.activation(th[:], s2[:], Act.Sqrt, bias=zero1[:])
            v.tensor_scalar(th[:], th[:], 1e-12, None, Al.max)
            rth = single.tile([B_PER, 1], f32)
            v.reciprocal(rth[:], th[:])
            u = single.tile([B_PER, 3], f32)
            v.tensor_scalar(u[:], wv[:], rth[:], None, Al.mult)
            # Pack [theta, theta + pi/2] into one [8,2] tile, range-reduce into
            # [-pi, pi] (Sin LUT domain; y -> y - 2pi*(y>pi) twice, since theta
            # can reach ~10), then a single Sin gives sin and cos together.
            th2 = single.tile([B_PER, 2], f32)
            v.tensor_copy(th2[:, 0:1], th[:])
            v.tensor_scalar(th2[:, 1:2], th[:], math.pi / 2, None, Al.add)
            mask1 = single.tile([B_PER, 2], f32)
            for _ in range(2):
                v.tensor_scalar(mask1[:], th2[:], math.pi, None, Al.is_gt)
                v.scalar_tensor_tensor(th2[:], mask1[:], -2.0 * math.pi, th2[:],
                                       Al.mult, Al.add)
            sc2 = single.tile([B_PER, 2], f32)
            sc.activation(sc2[:], th2[:], Act.Sin, bias=zero1[:])
            sint = sc2[:, 0:1]
            cost = sc2[:, 1:2]
            omc = single.tile([B_PER, 1], f32)
            v.tensor_scalar(omc[:], cost[:], -1.0, 1.0, Al.mult, Al.add)

            uu = single.tile([B_PER, 9], f32)
            u_i = u[:].unsqueeze(2).broadcast_to([B_PER, 3, 3])
            u_j = u[:].unsqueeze(1).broadcast_to([B_PER, 3, 3])
            v.tensor_tensor(uu[:].rearrange("b (i j) -> b i j", i=3), u_i, u_j, Al.mult)

            kk = single.tile([B_PER, 9], f32)
            v.memset(kk[:], 0.0)
            for col, (src, sgn) in {1: (2, -1.0), 2: (1, 1.0), 3: (2, 1.0),
                                    5: (0, -1.0), 6: (1, -1.0), 7: (0, 1.0)}.items():
                v.tensor_scalar(kk[:, col : col + 1], u[:, src : src + 1], sgn, None, Al.mult)

            t9 = single.tile([B_PER, 9], f32)
            v.tensor_scalar(t9[:], uu[:], omc[:], None, Al.mult)
            dr = single.tile([B_PER, 9], f32)
            v.scalar_tensor_tensor(dr[:], kk[:], sint[:], t9[:], Al.mult, Al.add)
            v.scalar_tensor_tensor(dr[:], eye3t[:], cost[:], dr[:], Al.mult, Al.add)

            # ---- dtv = dR @ xi[3:]; R_new = pR@dR; t_new = pt - pR@dtv --
            p9 = single.tile([B_PER, 9], f32)
            drv = dr[:].rearrange("b (i j) -> b i j", i=3)
            xi2 = xi[:, 3:6].unsqueeze(1).broadcast_to([B_PER, 3, 3])
            v.tensor_tensor(p9[:].rearrange("b (i j) -> b i j", i=3), drv, xi2, Al.mult)
            dtv = single.tile([B_PER, 3], f32)
            v.tensor_reduce(dtv[:], p9[:].rearrange("b (i j) -> b i j", i=3),
                            axis=mybir.AxisListType.X, op=Al.add)

            p27 = single.tile([B_PER, 27], f32)
            pr_v = prt[:].rearrange("b (i j) -> b i j", i=3).unsqueeze(2).broadcast_to([B_PER, 3, 3, 3])
            dr_v = dr[:].rearrange("b (j k) -> b k j", j=3).unsqueeze(1).broadcast_to([B_PER, 3, 3, 3])
            v.tensor_tensor(p27[:].rearrange("b (i k j) -> b i k j", i=3, k=3), pr_v, dr_v, Al.mult)
            out12 = single.tile([B_PER, 12], f32)
            v.tensor_reduce(out12[:, 0:9], p27[:].rearrange("b (i k j) -> b i k j", i=3, k=3),
                            axis=mybir.AxisListType.X, op=Al.add)

            prv3 = prt[:].rearrange("b (i j) -> b i j", i=3)
            dtb = dtv[:].unsqueeze(1).broadcast_to([B_PER, 3, 3])
            v.tensor_tensor(p9[:].rearrange("b (i j) -> b i j", i=3), prv3, dtb, Al.mult)
            tmp3 = single.tile([B_PER, 3], f32)
            v.tensor_reduce(tmp3[:], p9[:].rearrange("b (i j) -> b i j", i=3),
                            axis=mybir.AxisListType.X, op=Al.add)
            v.tensor_sub(out12[:, 9:12], ptt[:], tmp3[:])

            nc.sync.dma_start(out_d, out12[:])

    nc.compile()
    return nc


def _get_nc():
    if "nc" not in _CACHE:
        _CACHE["nc"] = _build_module()
    return _CACHE["nc"]


def _make_in_maps(JtJ, Jt, weights, R, pose_R, pose_t):
    eye6 = np.broadcast_to(np.eye(6, dtype=np.float32).reshape(1, 36), (B_PER, 36)).copy()
    eye3 = np.broadcast_to(np.eye(3, dtype=np.float32).reshape(1, 9), (B_PER, 9)).copy()
    in_maps = []
    for c in range(N_CORES):
        s = slice(c * B_PER, (c + 1) * B_PER)
        in_maps.append(
            {
                "jt": np.ascontiguousarray(Jt[s], dtype=np.float32),
                "w": np.ascontiguousarray(weights[s], dtype=np.float32).reshape(B_PER, N),
                "r": np.ascontiguousarray(R[s], dtype=np.float32).reshape(B_PER, N),
                "jtj": np.ascontiguousarray(JtJ[s], dtype=np.float32).reshape(B_PER, 36),
                "pr": np.ascontiguousarray(pose_R[s], dtype=np.float32).reshape(B_PER, 9),
                "pt": np.ascontiguousarray(pose_t[s], dtype=np.float32),
                "eye6": eye6,
                "eye3": eye3,
            }
        )
    return in_maps


def run(inputs, trace=False, **kw):
    """Run the bass kernel on the 8 cores. Returns ((R_new, t_new), results)."""
    from concourse import bass_utils

    nc = _get_nc()
    in_maps = _make_in_maps(
        inputs["JtJ"], inputs["Jt"], inputs["weights"], inputs["R"],
        inputs["pose_R"], inputs["pose_t"],
    )
    res = bass_utils.run_bass_kernel_spmd(
        nc, in_maps, core_ids=list(range(N_CORES)), trace=trace, **kw
    )
    out = np.concatenate([res.results[c]["out12"] for c in range(N_CORES)])
    rn = np.ascontiguousarray(out[:, 0:9], dtype=np.float32).reshape(B, 3, 3)
    tn = np.ascontiguousarray(out[:, 9:12], dtype=np.float32)
    return (rn, tn), res


def kernel(JtJ, Jt, weights, R, pose_R, pose_t, invD0=None, invD1=None,
           x0=None, x1=None, K=None):
    (rn, tn), _ = run(dict(JtJ=JtJ, Jt=Jt, weights=weights, R=R,
                           pose_R=pose_R, pose_t=pose_t))
    return rn, tn
